# revision 13
# baseline (speedup 1.0000x reference)
"""Multi-head attention + output projection on 8 Trainium2 NeuronCores.

Problem (hardcoded): B=2, N=S=2048, DIM=1024, 8 heads, head_dim=128, fp32.
  out = softmax(Q K^T / sqrt(128)) V  -> reshape -> @ proj_w.T + proj_b

Sharding: data parallel on batch (2) x tensor parallel on heads (4 groups of
2 heads).  Each core computes attention for its 2 heads plus the partial
output projection restricted to its heads' columns; the host sums the 4
partial projections per batch and adds the bias.

Per-core kernel (matmul operands fp16, accumulation fp32 PSUM):
  S^T = K @ Q^T per 128-row s-chunk with s on partitions (softmax needs no
  on-chip transpose of P), exp on ScalarE in 4-chunk batches (PSUM->SBUF,
  scale pre-applied to Q on host), out^T = V^T @ expS^T accumulated in PSUM.
  Row sums: chunk-accumulate expS^T on VectorE, one all-ones [128x128]
  matmul broadcasts the partition colsum, reciprocal_approx_fast + multiply
  normalizes out^T.  Projection Y = X @ W^T is interleaved per 512-row
  block so its matmuls fill TensorE stalls during exp-paced stretches.
"""

import sys

sys.path.insert(0, "/opt/trn_rl_repo")

import numpy as np

import concourse.bass as bass  # noqa: F401  (engine namespaces live on nc)
import concourse.mybir as mybir
import concourse.tile as tile
from concourse import bacc
from concourse.bass_utils import run_bass_kernel_spmd

B = 2
N = 2048
S = 2048
DIM = 1024
NUM_HEADS = 8
HD = 128
N_CORES = 8
HEADS_PER_CORE = 2  # 4-way head parallel x 2-way batch parallel
HG = DIM // (NUM_HEADS // HEADS_PER_CORE)  # 256 dims per core
P = 128
SC = S // P  # 16 s-chunks
NB = 512  # query-column block
NQ = N // NB
GC = 4  # s-chunks per exp group
F32 = mybir.dt.float32
F16 = mybir.dt.float16

_nc_cache = {}


def _build():
    nc = bacc.Bacc(None, target_bir_lowering=False, debug=False, num_devices=1)

    qt = nc.dram_tensor("qt", [HG, N], F16, kind="ExternalInput").ap()
    kt = nc.dram_tensor("kt", [HG, S], F16, kind="ExternalInput").ap()
    v = nc.dram_tensor("v", [S, HG], F16, kind="ExternalInput").ap()
    wt = nc.dram_tensor("wt", [HG, DIM], F16, kind="ExternalInput").ap()
    out = nc.dram_tensor("out", [N, DIM], F32, kind="ExternalOutput").ap()

    EXPF = mybir.ActivationFunctionType.Exp

    with tile.TileContext(nc) as tc:
        with (
            tc.tile_pool(name="persist", bufs=1) as persist,
            tc.tile_pool(name="e_pool", bufs=3) as e_pool,
            tc.tile_pool(name="a_pool", bufs=2) as a_pool,
            tc.tile_pool(name="small", bufs=2) as small,
            tc.tile_pool(name="y_pool", bufs=3) as y_pool,
            tc.tile_pool(name="s_ps_pool", bufs=1, space="PSUM") as s_ps_pool,
            tc.tile_pool(name="acc_ps_pool", bufs=3, space="PSUM") as acc_ps_pool,
            tc.tile_pool(name="rb_ps_pool", bufs=1, space="PSUM") as rb_ps_pool,
        ):
            # Resident inputs: one big DMA each.
            qt_sb = persist.tile([P, HEADS_PER_CORE, N], F16)
            nc.sync.dma_start(out=qt_sb, in_=qt.rearrange("(h p) n -> p h n", p=P))
            kt_sb = persist.tile([P, HEADS_PER_CORE, S], F16)
            nc.sync.dma_start(out=kt_sb, in_=kt.rearrange("(h p) s -> p h s", p=P))
            v_sb = persist.tile([P, HEADS_PER_CORE, SC, HD], F16)
            nc.sync.dma_start(
                out=v_sb, in_=v.rearrange("(c p) (h d) -> p h c d", p=P, h=HEADS_PER_CORE)
            )
            wt_sb = persist.tile([P, HEADS_PER_CORE, DIM], F16)
            nc.sync.dma_start(out=wt_sb, in_=wt.rearrange("(h p) o -> p h o", p=P))

            ones_dram = nc.inline_tensor(np.ones((P, P), np.float16), name="ones_const")
            ones_mat = persist.tile([P, P], F16)
            nc.sync.dma_start(out=ones_mat, in_=ones_dram.ap())

            # X^T: normalized attention outputs, head-dim on partitions.
            xt_sb = persist.tile([P, HEADS_PER_CORE, N], F16)

            for nq in range(NQ):
                nsl = slice(nq * NB, (nq + 1) * NB)
                for h in range(HEADS_PER_CORE):
                    q_blk = qt_sb[:, h, nsl]
                    o_ps = acc_ps_pool.tile([P, NB], F32, tag="acc")
                    a2 = a_pool.tile([P, GC, NB], F16, tag="a2")
                    for g in range(SC // GC):
                        s_ps = s_ps_pool.tile([P, GC, NB], F32, tag="s")
                        for j in range(GC):
                            si = GC * g + j
                            nc.tensor.matmul(
                                s_ps[:, j, :],
                                kt_sb[:, h, si * P : (si + 1) * P],
                                q_blk,
                                start=True,
                                stop=True,
                            )
                        e_t = e_pool.tile([P, GC, NB], F16, tag="e")
                        nc.scalar.activation(out=e_t, in_=s_ps, func=EXPF)
                        for j in range(GC):
                            si = GC * g + j
                            nc.tensor.matmul(
                                o_ps,
                                v_sb[:, h, si, :],
                                e_t[:, j, :],
                                start=(si == 0),
                                stop=(si == SC - 1),
                            )
                        with nc.allow_low_precision(
                            reason="fp16 rowsum partials; r ~2e3, fp16 keeps ~3e-4 rel"
                        ):
                            if g == 0:
                                nc.vector.tensor_copy(a2, e_t)
                            else:
                                nc.vector.tensor_add(a2, a2, e_t)
                    a15 = a_pool.tile([P, 2, NB], F16, tag="a15")
                    a1 = a_pool.tile([P, NB], F16, tag="a1")
                    with nc.allow_low_precision(reason="fp16 rowsum partials"):
                        nc.vector.tensor_add(a15, a2[:, 0:2, :], a2[:, 2:4, :])
                        nc.vector.tensor_add(a1, a15[:, 0, :], a15[:, 1, :])
                    # all-ones matmul: colsum over partitions, broadcast to 128
                    rb_ps = rb_ps_pool.tile([P, NB], F32, tag="rb")
                    nc.tensor.matmul(rb_ps, ones_mat, a1, start=True, stop=True)
                    recip = small.tile([P, NB], F32, tag="recip")
                    nc.vector.reciprocal_approx_fast(out=recip, in_=rb_ps)
                    with nc.allow_low_precision(reason="fp16 attention output grid"):
                        nc.vector.tensor_mul(xt_sb[:, h, nsl], o_ps, recip)

                # Partial projection for this 512-row block (both heads ready).
                for t in range(NB // P):
                    nt = nq * (NB // P) + t
                    y_sb = y_pool.tile([P, DIM], F32, tag="y")
                    for ot in range(2):
                        y_ps = acc_ps_pool.tile([P, NB], F32, tag="acc")
                        for h in range(HEADS_PER_CORE):
                            nc.tensor.matmul(
                                y_ps,
                                xt_sb[:, h, nt * P : (nt + 1) * P],
                                wt_sb[:, h, ot * NB : (ot + 1) * NB],
                                start=(h == 0),
                                stop=(h == HEADS_PER_CORE - 1),
                            )
                        if ot == 0:
                            nc.vector.tensor_copy(y_sb[:, ot * NB : (ot + 1) * NB], y_ps)
                        else:
                            nc.scalar.copy(y_sb[:, ot * NB : (ot + 1) * NB], y_ps)
                    nc.sync.dma_start(out=out[nt * P : (nt + 1) * P, :], in_=y_sb)

    nc.compile()
    return nc


def kernel(query, key, value, proj_w, proj_b):
    if "nc" not in _nc_cache:
        _nc_cache["nc"] = _build()
    nc = _nc_cache["nc"]

    scale = float(HD) ** -0.5
    wt_full = np.ascontiguousarray(proj_w.T.astype(np.float32))  # [in, out]
    in_maps = []
    for core in range(N_CORES):
        b, hg = divmod(core, N_CORES // B)
        sl = slice(hg * HG, (hg + 1) * HG)
        in_maps.append(
            {
                "qt": np.ascontiguousarray((query[b].T[sl] * scale), dtype=np.float16),
                "kt": np.ascontiguousarray(key[b].T[sl], dtype=np.float16),
                "v": np.ascontiguousarray(value[b][:, sl], dtype=np.float16),
                "wt": np.ascontiguousarray(wt_full[sl], dtype=np.float16),
            }
        )

    res = run_bass_kernel_spmd(nc, in_maps, list(range(N_CORES)))

    out = np.zeros((B, N, DIM), dtype=np.float32)
    for core in range(N_CORES):
        b = core // (N_CORES // B)
        out[b] += res.results[core]["out"]
    out += proj_b.astype(np.float32)
    return out


# revision 15
# speedup vs baseline: 1.1794x; 1.1794x over previous
"""Multi-head attention + output projection on 8 Trainium2 NeuronCores.

Problem (hardcoded): B=2, N=S=2048, DIM=1024, 8 heads, head_dim=128, fp32.
  out = softmax(Q K^T / sqrt(128)) V  -> reshape -> @ proj_w.T + proj_b

Sharding: data parallel on batch (2) x tensor parallel on heads (4 groups of
2 heads).  Each core computes attention for its 2 heads plus the partial
output projection restricted to its heads' columns; the host sums the 4
partial projections per batch and adds the bias.

Per-core kernel (matmul operands fp16, accumulation fp32 PSUM):
  S^T = K @ Q^T per 128-row s-chunk with s on partitions (softmax needs no
  on-chip transpose of P), exp on ScalarE in 4-chunk batches (PSUM->SBUF,
  scale pre-applied to Q on host), out^T = V^T @ expS^T accumulated in PSUM.
  Row sums: chunk-accumulate expS^T on VectorE, one all-ones [128x128]
  matmul broadcasts the partition colsum, reciprocal_approx_fast + multiply
  normalizes out^T.  Projection Y = X @ W^T is interleaved per 512-row
  block so its matmuls fill TensorE stalls during exp-paced stretches.
"""

import sys

sys.path.insert(0, "/opt/trn_rl_repo")

import numpy as np

import concourse.bass as bass  # noqa: F401  (engine namespaces live on nc)
import concourse.mybir as mybir
import concourse.tile as tile
from concourse import bacc
from concourse.bass_utils import run_bass_kernel_spmd

B = 2
N = 2048
S = 2048
DIM = 1024
NUM_HEADS = 8
HD = 128
N_CORES = 8
HEADS_PER_CORE = 2  # 4-way head parallel x 2-way batch parallel
HG = DIM // (NUM_HEADS // HEADS_PER_CORE)  # 256 dims per core
P = 128
SC = S // P  # 16 s-chunks
NB = 512  # query-column block
NQ = N // NB
GC = 2  # s-chunks per exp group
F32 = mybir.dt.float32
F16 = mybir.dt.float16

_nc_cache = {}


def _build():
    nc = bacc.Bacc(None, target_bir_lowering=False, debug=False, num_devices=1)

    qt = nc.dram_tensor("qt", [HG, N], F16, kind="ExternalInput").ap()
    kt = nc.dram_tensor("kt", [HG, S], F16, kind="ExternalInput").ap()
    v = nc.dram_tensor("v", [S, HG], F16, kind="ExternalInput").ap()
    wt = nc.dram_tensor("wt", [HG, DIM], F16, kind="ExternalInput").ap()
    out = nc.dram_tensor("out", [N, DIM], F32, kind="ExternalOutput").ap()

    EXPF = mybir.ActivationFunctionType.Exp

    with tile.TileContext(nc) as tc:
        with (
            tc.tile_pool(name="persist", bufs=1) as persist,
            tc.tile_pool(name="e_pool", bufs=3) as e_pool,
            tc.tile_pool(name="a_pool", bufs=2) as a_pool,
            tc.tile_pool(name="small", bufs=2) as small,
            tc.tile_pool(name="y_pool", bufs=3) as y_pool,
            tc.tile_pool(name="s_ps_pool", bufs=3, space="PSUM") as s_ps_pool,
            tc.tile_pool(name="acc_ps_pool", bufs=2, space="PSUM") as acc_ps_pool,
        ):
            # Resident inputs: one big DMA each.
            qt_sb = persist.tile([P, HEADS_PER_CORE, N], F16)
            nc.sync.dma_start(out=qt_sb, in_=qt.rearrange("(h p) n -> p h n", p=P))
            kt_sb = persist.tile([P, HEADS_PER_CORE, S], F16)
            nc.sync.dma_start(out=kt_sb, in_=kt.rearrange("(h p) s -> p h s", p=P))
            v_sb = persist.tile([P, HEADS_PER_CORE, SC, HD], F16)
            nc.sync.dma_start(
                out=v_sb, in_=v.rearrange("(c p) (h d) -> p h c d", p=P, h=HEADS_PER_CORE)
            )
            wt_sb = persist.tile([P, HEADS_PER_CORE, DIM], F16)
            nc.sync.dma_start(out=wt_sb, in_=wt.rearrange("(h p) o -> p h o", p=P))

            ones_dram = nc.inline_tensor(np.ones((P, P), np.float16), name="ones_const")
            ones_mat = persist.tile([P, P], F16)
            nc.sync.dma_start(out=ones_mat, in_=ones_dram.ap())

            # X^T: normalized attention outputs, head-dim on partitions.
            xt_sb = persist.tile([P, HEADS_PER_CORE, N], F16)

            for nq in range(NQ):
                nsl = slice(nq * NB, (nq + 1) * NB)
                for h in range(HEADS_PER_CORE):
                    q_blk = qt_sb[:, h, nsl]
                    o_ps = acc_ps_pool.tile([P, NB], F32, tag="acc")
                    a2 = a_pool.tile([P, GC, NB], F16, tag="a2")
                    a2g = a_pool.tile([P, GC, NB], F16, tag="a2g")
                    for g in range(SC // GC):
                        s_ps = s_ps_pool.tile([P, GC, NB], F32, tag="s")
                        for j in range(GC):
                            si = GC * g + j
                            nc.tensor.matmul(
                                s_ps[:, j, :],
                                kt_sb[:, h, si * P : (si + 1) * P],
                                q_blk,
                                start=True,
                                stop=True,
                            )
                        e_t = e_pool.tile([P, GC, NB], F16, tag="e")
                        nc.scalar.activation(out=e_t, in_=s_ps, func=EXPF)
                        for j in range(GC):
                            si = GC * g + j
                            nc.tensor.matmul(
                                o_ps,
                                v_sb[:, h, si, :],
                                e_t[:, j, :],
                                start=(si == 0),
                                stop=(si == SC - 1),
                            )
                        with nc.allow_low_precision(
                            reason="fp16 rowsum partials; r ~2e3, fp16 keeps ~3e-4 rel"
                        ):
                            acc = a2 if g % 2 == 0 else a2g
                            if g < 2:
                                nc.vector.tensor_copy(acc, e_t)
                            else:
                                nc.vector.tensor_add(acc, acc, e_t)
                    a15 = a_pool.tile([P, GC, NB], F16, tag="a15")
                    a1 = a_pool.tile([P, NB], F16, tag="a1")
                    with nc.allow_low_precision(reason="fp16 rowsum partials"):
                        nc.vector.tensor_add(a15, a2, a2g)
                        nc.vector.tensor_add(a1, a15[:, 0, :], a15[:, 1, :])
                    # all-ones matmul: colsum over partitions, broadcast to 128
                    rb_full = s_ps_pool.tile([P, GC, NB], F32, tag="s")
                    rb_ps = rb_full[:, 0, :]
                    nc.tensor.matmul(rb_ps, ones_mat, a1, start=True, stop=True)
                    recip = small.tile([P, NB], F32, tag="recip")
                    nc.vector.reciprocal_approx_fast(out=recip, in_=rb_ps)
                    with nc.allow_low_precision(reason="fp16 attention output grid"):
                        nc.vector.tensor_mul(xt_sb[:, h, nsl], o_ps, recip)

                # Partial projection for this 512-row block (both heads ready).
                for t in range(NB // P):
                    nt = nq * (NB // P) + t
                    y_sb = y_pool.tile([P, DIM], F32, tag="y")
                    for ot in range(2):
                        y_ps = acc_ps_pool.tile([P, NB], F32, tag="acc")
                        for h in range(HEADS_PER_CORE):
                            nc.tensor.matmul(
                                y_ps,
                                xt_sb[:, h, nt * P : (nt + 1) * P],
                                wt_sb[:, h, ot * NB : (ot + 1) * NB],
                                start=(h == 0),
                                stop=(h == HEADS_PER_CORE - 1),
                            )
                        nc.vector.tensor_copy(y_sb[:, ot * NB : (ot + 1) * NB], y_ps)
                    nc.sync.dma_start(out=out[nt * P : (nt + 1) * P, :], in_=y_sb)

    nc.compile()
    return nc


def kernel(query, key, value, proj_w, proj_b):
    if "nc" not in _nc_cache:
        _nc_cache["nc"] = _build()
    nc = _nc_cache["nc"]

    scale = float(HD) ** -0.5
    wt_full = np.ascontiguousarray(proj_w.T.astype(np.float32))  # [in, out]
    in_maps = []
    for core in range(N_CORES):
        b, hg = divmod(core, N_CORES // B)
        sl = slice(hg * HG, (hg + 1) * HG)
        in_maps.append(
            {
                "qt": np.ascontiguousarray((query[b].T[sl] * scale), dtype=np.float16),
                "kt": np.ascontiguousarray(key[b].T[sl], dtype=np.float16),
                "v": np.ascontiguousarray(value[b][:, sl], dtype=np.float16),
                "wt": np.ascontiguousarray(wt_full[sl], dtype=np.float16),
            }
        )

    res = run_bass_kernel_spmd(nc, in_maps, list(range(N_CORES)))

    out = np.zeros((B, N, DIM), dtype=np.float32)
    for core in range(N_CORES):
        b = core // (N_CORES // B)
        out[b] += res.results[core]["out"]
    out += proj_b.astype(np.float32)
    return out


# revision 16
# speedup vs baseline: 1.4398x; 1.2208x over previous
"""Multi-head attention + output projection on 8 Trainium2 NeuronCores.

Problem (hardcoded): B=2, N=S=2048, DIM=1024, 8 heads, head_dim=128, fp32.
  out = softmax(Q K^T / sqrt(128)) V  -> reshape -> @ proj_w.T + proj_b

Sharding: data parallel on batch (2) x tensor parallel on heads (4 groups of
2 heads).  Each core computes attention for its 2 heads plus the partial
output projection restricted to its heads' columns; the host sums the 4
partial projections per batch and adds the bias.

Per-core kernel (matmul operands fp16, accumulation fp32 PSUM):
  S^T = K @ Q^T per 128-row s-chunk with s on partitions (softmax needs no
  on-chip transpose of P), exp on ScalarE in 4-chunk batches (PSUM->SBUF,
  scale pre-applied to Q on host), out^T = V^T @ expS^T accumulated in PSUM.
  Row sums: chunk-accumulate expS^T on VectorE, one all-ones [128x128]
  matmul broadcasts the partition colsum, reciprocal_approx_fast + multiply
  normalizes out^T.  Projection Y = X @ W^T is interleaved per 512-row
  block so its matmuls fill TensorE stalls during exp-paced stretches.
"""

import sys

sys.path.insert(0, "/opt/trn_rl_repo")

import numpy as np

import concourse.bass as bass  # noqa: F401  (engine namespaces live on nc)
import concourse.mybir as mybir
import concourse.tile as tile
from concourse import bacc
from concourse.bass_utils import run_bass_kernel_spmd

B = 2
N = 2048
S = 2048
DIM = 1024
NUM_HEADS = 8
HD = 128
N_CORES = 8
HEADS_PER_CORE = 2  # 4-way head parallel x 2-way batch parallel
HG = DIM // (NUM_HEADS // HEADS_PER_CORE)  # 256 dims per core
P = 128
SC = S // P  # 16 s-chunks
NB = 512  # query-column block
NQ = N // NB
GC = 2  # s-chunks per exp group
F32 = mybir.dt.float32
F16 = mybir.dt.float16

_nc_cache = {}


def _build():
    nc = bacc.Bacc(None, target_bir_lowering=False, debug=False, num_devices=1)

    qt = nc.dram_tensor("qt", [HG, N], F16, kind="ExternalInput").ap()
    kt = nc.dram_tensor("kt", [HG, S], F16, kind="ExternalInput").ap()
    v = nc.dram_tensor("v", [S, HG], F16, kind="ExternalInput").ap()
    wt = nc.dram_tensor("wt", [HG, DIM], F16, kind="ExternalInput").ap()
    out = nc.dram_tensor("out", [N, DIM], F32, kind="ExternalOutput").ap()

    EXPF = mybir.ActivationFunctionType.Exp

    with tile.TileContext(nc) as tc:
        with (
            tc.tile_pool(name="persist", bufs=1) as persist,
            tc.tile_pool(name="e_pool", bufs=6) as e_pool,
            tc.tile_pool(name="a_pool", bufs=3) as a_pool,
            tc.tile_pool(name="small", bufs=3) as small,
            tc.tile_pool(name="y_pool", bufs=3) as y_pool,
            tc.tile_pool(name="s_ps_pool", bufs=3, space="PSUM") as s_ps_pool,
            tc.tile_pool(name="acc_ps_pool", bufs=2, space="PSUM") as acc_ps_pool,
        ):
            # Resident inputs, sliced per head so the first QK starts early.
            qt_sb = persist.tile([P, HEADS_PER_CORE, N], F16)
            kt_sb = persist.tile([P, HEADS_PER_CORE, S], F16)
            v_sb = persist.tile([P, HEADS_PER_CORE, SC, HD], F16)
            wt_sb = persist.tile([P, HEADS_PER_CORE, DIM], F16)
            qt_r = qt.rearrange("(h p) n -> p h n", p=P)
            kt_r = kt.rearrange("(h p) s -> p h s", p=P)
            v_r = v.rearrange("(c p) (h d) -> p h c d", p=P, h=HEADS_PER_CORE)
            wt_r = wt.rearrange("(h p) o -> p h o", p=P)
            ones_dram = nc.inline_tensor(np.ones((P, P), np.float16), name="ones_const")
            ones_mat = persist.tile([P, P], F16)
            nc.sync.dma_start(out=kt_sb[:, 0], in_=kt_r[:, 0])
            nc.sync.dma_start(out=qt_sb[:, 0], in_=qt_r[:, 0])
            nc.sync.dma_start(out=v_sb[:, 0], in_=v_r[:, 0])
            nc.sync.dma_start(out=ones_mat, in_=ones_dram.ap())
            nc.sync.dma_start(out=kt_sb[:, 1], in_=kt_r[:, 1])
            nc.sync.dma_start(out=qt_sb[:, 1], in_=qt_r[:, 1])
            nc.sync.dma_start(out=v_sb[:, 1], in_=v_r[:, 1])
            nc.sync.dma_start(out=wt_sb[:, 0], in_=wt_r[:, 0])
            nc.sync.dma_start(out=wt_sb[:, 1], in_=wt_r[:, 1])

            # X^T: normalized attention outputs, head-dim on partitions.
            xt_sb = persist.tile([P, HEADS_PER_CORE, N], F16)

            for nq in range(NQ):
                nsl = slice(nq * NB, (nq + 1) * NB)
                for h in range(HEADS_PER_CORE):
                    q_blk = qt_sb[:, h, nsl]
                    o_ps = acc_ps_pool.tile([P, NB], F32, tag="acc")
                    a2 = a_pool.tile([P, GC, NB], F16, tag="a2")
                    a2g = a_pool.tile([P, GC, NB], F16, tag="a2g")
                    for g in range(SC // GC):
                        s_ps = s_ps_pool.tile([P, GC, NB], F32, tag="s")
                        for j in range(GC):
                            si = GC * g + j
                            nc.tensor.matmul(
                                s_ps[:, j, :],
                                kt_sb[:, h, si * P : (si + 1) * P],
                                q_blk,
                                start=True,
                                stop=True,
                            )
                        e_t = e_pool.tile([P, GC, NB], F16, tag="e")
                        nc.scalar.activation(out=e_t, in_=s_ps, func=EXPF)
                        for j in range(GC):
                            si = GC * g + j
                            nc.tensor.matmul(
                                o_ps,
                                v_sb[:, h, si, :],
                                e_t[:, j, :],
                                start=(si == 0),
                                stop=(si == SC - 1),
                            )
                        with nc.allow_low_precision(
                            reason="fp16 rowsum partials; r ~2e3, fp16 keeps ~3e-4 rel"
                        ):
                            acc = a2 if g % 2 == 0 else a2g
                            if g < 2:
                                nc.vector.tensor_copy(acc, e_t)
                            else:
                                nc.vector.tensor_add(acc, acc, e_t)
                    # all-ones matmuls: colsum over partitions of the four
                    # partial-sum tiles, broadcast to 128, accumulated in PSUM
                    rb_full = s_ps_pool.tile([P, GC, NB], F32, tag="s")
                    rb_ps = rb_full[:, 0, :]
                    parts = [a2[:, 0, :], a2[:, 1, :], a2g[:, 0, :], a2g[:, 1, :]]
                    for pi, part in enumerate(parts):
                        nc.tensor.matmul(
                            rb_ps, ones_mat, part,
                            start=(pi == 0), stop=(pi == len(parts) - 1),
                        )
                    recip = small.tile([P, NB], F32, tag="recip")
                    nc.vector.reciprocal_approx_fast(out=recip, in_=rb_ps)
                    with nc.allow_low_precision(reason="fp16 attention output grid"):
                        nc.vector.tensor_mul(xt_sb[:, h, nsl], o_ps, recip)

                # Partial projection for this 512-row block (both heads ready).
                for t in range(NB // P):
                    nt = nq * (NB // P) + t
                    y_sb = y_pool.tile([P, DIM], F32, tag="y")
                    for ot in range(2):
                        y_ps = acc_ps_pool.tile([P, NB], F32, tag="acc")
                        for h in range(HEADS_PER_CORE):
                            nc.tensor.matmul(
                                y_ps,
                                xt_sb[:, h, nt * P : (nt + 1) * P],
                                wt_sb[:, h, ot * NB : (ot + 1) * NB],
                                start=(h == 0),
                                stop=(h == HEADS_PER_CORE - 1),
                            )
                        nc.vector.tensor_copy(y_sb[:, ot * NB : (ot + 1) * NB], y_ps)
                    nc.sync.dma_start(out=out[nt * P : (nt + 1) * P, :], in_=y_sb)

    nc.compile()
    return nc


def kernel(query, key, value, proj_w, proj_b):
    if "nc" not in _nc_cache:
        _nc_cache["nc"] = _build()
    nc = _nc_cache["nc"]

    scale = float(HD) ** -0.5
    wt_full = np.ascontiguousarray(proj_w.T.astype(np.float32))  # [in, out]
    in_maps = []
    for core in range(N_CORES):
        b, hg = divmod(core, N_CORES // B)
        sl = slice(hg * HG, (hg + 1) * HG)
        in_maps.append(
            {
                "qt": np.ascontiguousarray((query[b].T[sl] * scale), dtype=np.float16),
                "kt": np.ascontiguousarray(key[b].T[sl], dtype=np.float16),
                "v": np.ascontiguousarray(value[b][:, sl], dtype=np.float16),
                "wt": np.ascontiguousarray(wt_full[sl], dtype=np.float16),
            }
        )

    res = run_bass_kernel_spmd(nc, in_maps, list(range(N_CORES)))

    out = np.zeros((B, N, DIM), dtype=np.float32)
    for core in range(N_CORES):
        b = core // (N_CORES // B)
        out[b] += res.results[core]["out"]
    out += proj_b.astype(np.float32)
    return out


# revision 17
# speedup vs baseline: 1.4549x; 1.0105x over previous
"""Multi-head attention + output projection on 8 Trainium2 NeuronCores.

Problem (hardcoded): B=2, N=S=2048, DIM=1024, 8 heads, head_dim=128, fp32.
  out = softmax(Q K^T / sqrt(128)) V  -> reshape -> @ proj_w.T + proj_b

Sharding: data parallel on batch (2) x tensor parallel on heads (4 groups of
2 heads).  Each core computes attention for its 2 heads plus the partial
output projection restricted to its heads' columns; the host sums the 4
partial projections per batch and adds the bias.

Per-core kernel (matmul operands fp16, accumulation fp32 PSUM):
  S^T = K @ Q^T per 128-row s-chunk with s on partitions (softmax needs no
  on-chip transpose of P), exp on ScalarE in 4-chunk batches (PSUM->SBUF,
  scale pre-applied to Q on host), out^T = V^T @ expS^T accumulated in PSUM.
  Row sums: chunk-accumulate expS^T on VectorE, one all-ones [128x128]
  matmul broadcasts the partition colsum, reciprocal_approx_fast + multiply
  normalizes out^T.  Projection Y = X @ W^T is interleaved per 512-row
  block so its matmuls fill TensorE stalls during exp-paced stretches.
"""

import sys

sys.path.insert(0, "/opt/trn_rl_repo")

import numpy as np

import concourse.bass as bass  # noqa: F401  (engine namespaces live on nc)
import concourse.mybir as mybir
import concourse.tile as tile
from concourse import bacc
from concourse.bass_utils import run_bass_kernel_spmd

B = 2
N = 2048
S = 2048
DIM = 1024
NUM_HEADS = 8
HD = 128
N_CORES = 8
HEADS_PER_CORE = 2  # 4-way head parallel x 2-way batch parallel
HG = DIM // (NUM_HEADS // HEADS_PER_CORE)  # 256 dims per core
P = 128
SC = S // P  # 16 s-chunks
NB = 512  # query-column block
NQ = N // NB
GC = 2  # s-chunks per exp group
F32 = mybir.dt.float32
F16 = mybir.dt.float16

_nc_cache = {}


def _build():
    nc = bacc.Bacc(None, target_bir_lowering=False, debug=False, num_devices=1)

    qt = nc.dram_tensor("qt", [HG, N], F16, kind="ExternalInput").ap()
    kt = nc.dram_tensor("kt", [HG, S], F16, kind="ExternalInput").ap()
    v = nc.dram_tensor("v", [S, HG], F16, kind="ExternalInput").ap()
    wt = nc.dram_tensor("wt", [HG, DIM], F16, kind="ExternalInput").ap()
    out = nc.dram_tensor("out", [N, DIM], F32, kind="ExternalOutput").ap()

    EXPF = mybir.ActivationFunctionType.Exp

    with tile.TileContext(nc) as tc:
        with (
            tc.tile_pool(name="persist", bufs=1) as persist,
            tc.tile_pool(name="e_pool", bufs=6) as e_pool,
            tc.tile_pool(name="a_pool", bufs=3) as a_pool,
            tc.tile_pool(name="small", bufs=3) as small,
            tc.tile_pool(name="y_pool", bufs=3) as y_pool,
            tc.tile_pool(name="s_ps_pool", bufs=3, space="PSUM") as s_ps_pool,
            tc.tile_pool(name="acc_ps_pool", bufs=2, space="PSUM") as acc_ps_pool,
        ):
            # Resident inputs, sliced per head so the first QK starts early.
            qt_sb = persist.tile([P, HEADS_PER_CORE, N], F16)
            kt_sb = persist.tile([P, HEADS_PER_CORE, S], F16)
            v_sb = persist.tile([P, HEADS_PER_CORE, SC, HD], F16)
            wt_sb = persist.tile([P, HEADS_PER_CORE, DIM], F16)
            qt_r = qt.rearrange("(h p) n -> p h n", p=P)
            kt_r = kt.rearrange("(h p) s -> p h s", p=P)
            v_r = v.rearrange("(c p) (h d) -> p h c d", p=P, h=HEADS_PER_CORE)
            wt_r = wt.rearrange("(h p) o -> p h o", p=P)
            ones_dram = nc.inline_tensor(np.ones((P, P), np.float16), name="ones_const")
            ones_mat = persist.tile([P, P], F16)
            nc.sync.dma_start(out=qt_sb[:, 0, 0:NB], in_=qt_r[:, 0, 0:NB])
            nc.sync.dma_start(out=kt_sb[:, 0, 0 : S // 2], in_=kt_r[:, 0, 0 : S // 2])
            nc.sync.dma_start(out=v_sb[:, 0, 0 : SC // 2], in_=v_r[:, 0, 0 : SC // 2])
            nc.sync.dma_start(out=kt_sb[:, 0, S // 2 :], in_=kt_r[:, 0, S // 2 :])
            nc.sync.dma_start(out=v_sb[:, 0, SC // 2 :], in_=v_r[:, 0, SC // 2 :])
            nc.sync.dma_start(out=ones_mat, in_=ones_dram.ap())
            nc.sync.dma_start(out=qt_sb[:, 0, NB:], in_=qt_r[:, 0, NB:])
            nc.sync.dma_start(out=kt_sb[:, 1], in_=kt_r[:, 1])
            nc.sync.dma_start(out=qt_sb[:, 1], in_=qt_r[:, 1])
            nc.sync.dma_start(out=v_sb[:, 1], in_=v_r[:, 1])
            nc.sync.dma_start(out=wt_sb[:, 0], in_=wt_r[:, 0])
            nc.sync.dma_start(out=wt_sb[:, 1], in_=wt_r[:, 1])

            # X^T: normalized attention outputs, head-dim on partitions.
            xt_sb = persist.tile([P, HEADS_PER_CORE, N], F16)

            for nq in range(NQ):
                nsl = slice(nq * NB, (nq + 1) * NB)
                for h in range(HEADS_PER_CORE):
                    q_blk = qt_sb[:, h, nsl]
                    o_ps = acc_ps_pool.tile([P, NB], F32, tag="acc")
                    a2 = a_pool.tile([P, GC, NB], F16, tag="a2")
                    a2g = a_pool.tile([P, GC, NB], F16, tag="a2g")
                    e_last = []
                    for g in range(SC // GC):
                        s_ps = s_ps_pool.tile([P, GC, NB], F32, tag="s")
                        for j in range(GC):
                            si = GC * g + j
                            nc.tensor.matmul(
                                s_ps[:, j, :],
                                kt_sb[:, h, si * P : (si + 1) * P],
                                q_blk,
                                start=True,
                                stop=True,
                            )
                        e_t = e_pool.tile([P, GC, NB], F16, tag="e")
                        nc.scalar.activation(out=e_t, in_=s_ps, func=EXPF)
                        for j in range(GC):
                            si = GC * g + j
                            nc.tensor.matmul(
                                o_ps,
                                v_sb[:, h, si, :],
                                e_t[:, j, :],
                                start=(si == 0),
                                stop=(si == SC - 1),
                            )
                        if g < 6:
                            with nc.allow_low_precision(
                                reason="fp16 rowsum partials; r ~2e3, fp16 keeps ~3e-4 rel"
                            ):
                                acc = a2 if g % 2 == 0 else a2g
                                if g < 2:
                                    nc.vector.tensor_copy(acc, e_t)
                                else:
                                    nc.vector.tensor_add(acc, acc, e_t)
                        else:
                            e_last.append(e_t)
                        if g == 5:
                            rb_ps = acc_ps_pool.tile([P, NB], F32, tag="acc")
                            for pi, part in enumerate(
                                [a2[:, 0, :], a2[:, 1, :], a2g[:, 0, :], a2g[:, 1, :]]
                            ):
                                nc.tensor.matmul(
                                    rb_ps, ones_mat, part, start=(pi == 0), stop=False
                                )
                    # tail: last two groups' exp tiles feed the rowsum directly
                    tail_parts = [e[:, j, :] for e in e_last for j in range(GC)]
                    for pi, part in enumerate(tail_parts):
                        nc.tensor.matmul(
                            rb_ps, ones_mat, part,
                            start=False, stop=(pi == len(tail_parts) - 1),
                        )
                    recip = small.tile([P, NB], F32, tag="recip")
                    nc.vector.reciprocal_approx_fast(out=recip, in_=rb_ps)
                    with nc.allow_low_precision(reason="fp16 attention output grid"):
                        nc.vector.tensor_mul(xt_sb[:, h, nsl], o_ps, recip)

                # Partial projection for this 512-row block (both heads ready).
                for t in range(NB // P):
                    nt = nq * (NB // P) + t
                    y_sb = y_pool.tile([P, DIM], F32, tag="y")
                    for ot in range(2):
                        y_ps = acc_ps_pool.tile([P, NB], F32, tag="acc")
                        for h in range(HEADS_PER_CORE):
                            nc.tensor.matmul(
                                y_ps,
                                xt_sb[:, h, nt * P : (nt + 1) * P],
                                wt_sb[:, h, ot * NB : (ot + 1) * NB],
                                start=(h == 0),
                                stop=(h == HEADS_PER_CORE - 1),
                            )
                        nc.vector.tensor_copy(y_sb[:, ot * NB : (ot + 1) * NB], y_ps)
                    nc.sync.dma_start(out=out[nt * P : (nt + 1) * P, :], in_=y_sb)

    nc.compile()
    return nc


def kernel(query, key, value, proj_w, proj_b):
    if "nc" not in _nc_cache:
        _nc_cache["nc"] = _build()
    nc = _nc_cache["nc"]

    scale = float(HD) ** -0.5
    wt_full = np.ascontiguousarray(proj_w.T.astype(np.float32))  # [in, out]
    in_maps = []
    for core in range(N_CORES):
        b, hg = divmod(core, N_CORES // B)
        sl = slice(hg * HG, (hg + 1) * HG)
        in_maps.append(
            {
                "qt": np.ascontiguousarray((query[b].T[sl] * scale), dtype=np.float16),
                "kt": np.ascontiguousarray(key[b].T[sl], dtype=np.float16),
                "v": np.ascontiguousarray(value[b][:, sl], dtype=np.float16),
                "wt": np.ascontiguousarray(wt_full[sl], dtype=np.float16),
            }
        )

    res = run_bass_kernel_spmd(nc, in_maps, list(range(N_CORES)))

    out = np.zeros((B, N, DIM), dtype=np.float32)
    for core in range(N_CORES):
        b = core // (N_CORES // B)
        out[b] += res.results[core]["out"]
    out += proj_b.astype(np.float32)
    return out
